# revision 2
# baseline (speedup 1.0000x reference)
"""CausalBiTrilinearBCNAttention Trainium2 kernel, v3.

Math (see _fold_weights): xp = x @ P (D x 448), causal-cumsum the last 256
columns, G = [g1|g2] from elementwise products, out = G @ A.T.

Sharding: 8 cores = 4 batches x 2 T-halves (TH=1024 tokens/core); the
second half's cumsum seed sxP = (sum_t x[:TH]) @ P_cum comes from the host
as a [1,256] fp16 row (zeros for first halves).

Dataflow: host-preswizzled DMA layouts (every descriptor contiguous per
partition), inputs streamed in consumption order across both HWDGE queues,
tokens processed in two 512-token waves so wave 0's tail (cumsum/ew/final/
output) overlaps wave 1's x DMA + xp matmuls. 2 PSUM banks hold xp
accumulators (tiles 0-1 of each wave stream behind the DMA, tiles 2-3
replay from SBUF). Cumsum carries: tile pair (2j,2j+1) accumulates
U^T xp_even / +ones x carry / +ONES^T xp_even for the odd tile, so row 127
of the odd tile's PSUM IS the next pair's carry - one [1,256] copy per
pair, no block-sum matmuls, no cross-engine chain.
"""

import numpy as np

import concourse.bass as bass
import concourse.tile as tile
from concourse import bacc, mybir
from concourse.bass_utils import run_bass_kernel_spmd

B, T, D, R = 4, 2048, 1024, 64
TH = T // 2          # tokens per core
NT = TH // 128       # 8 token tiles per core
ND = D // 128        # 8 d chunks
PCOLS = 448          # 7 * R
CUM0 = 192           # start of cumsum group in P's columns
NCUM = 256           # cumsum group width

F32 = mybir.dt.float32
F16 = mybir.dt.float16


def build_nc():
    nc = bacc.Bacc(None, target_bir_lowering=False)

    xS = nc.dram_tensor("xS", [128, ND, TH], F16, kind="ExternalInput")
    PS = nc.dram_tensor("PS", [128, ND, PCOLS], F16, kind="ExternalInput")
    AT = nc.dram_tensor("AT", [128, D], F16, kind="ExternalInput")
    sxP = nc.dram_tensor("sxP", [1, NCUM], F16, kind="ExternalInput")
    invc = nc.dram_tensor("invc", [128, NT], F32, kind="ExternalInput")
    outT = nc.dram_tensor("outT", [D, TH], F16, kind="ExternalOutput")

    from concourse.masks import make_identity, make_upper_triangular

    with tile.TileContext(nc) as tc:
        with tc.tile_pool(name="consts", bufs=1) as consts, \
             tc.tile_pool(name="big", bufs=1) as big, \
             tc.tile_pool(name="ps", bufs=2, space="PSUM") as ps:

            # ---- SBUF tiles ----
            xT_sb = big.tile([128, ND, TH], F16)
            P_sb = consts.tile([128, ND, PCOLS], F16)
            AT_sb = consts.tile([128, D], F16)
            invc_sb = consts.tile([128, NT], F32)
            carry_sb = big.tile([32, 4, NCUM], F16)  # carry on row 31
            xp_sb = big.tile([128, NT, PCOLS], F16)
            cum_sb = big.tile([128, NT, NCUM], F16)
            G_sb = big.tile([128, NT, 128], F16)
            GT_sb = big.tile([128, TH], F16)
            m2_sb = big.tile([128, NT, 64], F16)
            o_stage = big.tile([128, ND, 512], F16)
            warm_sb = consts.tile([128, 448], F16)

            # ---- DMA issue: consumption order across both queues ----
            # q1 (sync):   sxP P01 xh0_01 P45 xh0_45 xh1_01 xh1_45 [+out]
            # q10 (scalar):invc P23 xh0_23 P67 xh0_67 xh1_23 xh1_67 AT [+out]
            nc.vector.memset(carry_sb, 0.0)
            nc.sync.dma_start(out=carry_sb[31:32, 0, :], in_=sxP[:, :])
            nc.scalar.dma_start(out=invc_sb, in_=invc[:, :])
            nc.sync.dma_start(out=P_sb[:, 0:2, :], in_=PS[:, 0:2, :])
            nc.scalar.dma_start(out=P_sb[:, 2:4, :], in_=PS[:, 2:4, :])
            nc.sync.dma_start(out=xT_sb[:, 0:2, 0:512], in_=xS[:, 0:2, 0:512])
            nc.scalar.dma_start(out=xT_sb[:, 2:4, 0:512],
                                in_=xS[:, 2:4, 0:512])
            nc.sync.dma_start(out=P_sb[:, 4:6, :], in_=PS[:, 4:6, :])
            nc.scalar.dma_start(out=P_sb[:, 6:8, :], in_=PS[:, 6:8, :])
            nc.sync.dma_start(out=xT_sb[:, 4:6, 0:512], in_=xS[:, 4:6, 0:512])
            nc.scalar.dma_start(out=xT_sb[:, 6:8, 0:512],
                                in_=xS[:, 6:8, 0:512])
            nc.sync.dma_start(out=xT_sb[:, 0:2, 512:1024],
                              in_=xS[:, 0:2, 512:1024])
            nc.scalar.dma_start(out=xT_sb[:, 2:4, 512:1024],
                                in_=xS[:, 2:4, 512:1024])
            nc.sync.dma_start(out=xT_sb[:, 4:6, 512:1024],
                              in_=xS[:, 4:6, 512:1024])
            nc.scalar.dma_start(out=xT_sb[:, 6:8, 512:1024],
                                in_=xS[:, 6:8, 512:1024])
            nc.scalar.dma_start(out=AT_sb, in_=AT[:, :])

            # ---- constants on idle engines ----
            nc.vector.memset(warm_sb, 0.0)
            U_sb = consts.tile([128, 128], F16)
            make_upper_triangular(nc, U_sb, val=1.0, diag=True)
            IDN_sb = consts.tile([128, 128], F16)
            make_identity(nc, IDN_sb)
            ONES_sb = consts.tile([128, 128], F16)
            nc.gpsimd.memset(ONES_sb, 1.0)
            # sel32: [32,128] with row 31 all-ones -- sel32^T @ carry32
            # broadcasts carry row 31 to all 128 output partitions
            sel32_sb = consts.tile([32, 128], F16)
            nc.gpsimd.memset(sel32_sb, 0.0)
            nc.gpsimd.affine_select(
                out=sel32_sb, in_=sel32_sb,
                compare_op=mybir.AluOpType.not_equal, fill=1.0,
                base=-31, pattern=[[0, 128]], channel_multiplier=1)

            # ---- PE warmup: ramp the PE while DMA streams in ----
            warm_ps = ps.tile([128, 512], F32, tag="fin", bufs=3)
            for i in range(14):
                nc.tensor.matmul(warm_ps[:, 0:448], warm_sb[:, 0:128],
                                 warm_sb, start=True, stop=True)
            # preload the scalar-engine activation table off the critical path
            actw_sb = consts.tile([128, 8], F16)
            nc.scalar.activation(actw_sb, warm_sb[:, 0:8],
                                 mybir.ActivationFunctionType.Copy)

            xp_ps = [None] * NT

            def emit_xp_stream(h, j):
                # xp matmuls for dk pair j of half h, first two tiles only
                for dk in (2 * j, 2 * j + 1):
                    for k in (4 * h, 4 * h + 1):
                        if dk == 0:
                            xp_ps[k] = ps.tile([128, PCOLS], F32, tag="xp",
                                               bufs=2, name=f"xp_{k}")
                        nc.tensor.matmul(
                            xp_ps[k],
                            xT_sb[:, dk, k * 128:(k + 1) * 128],
                            P_sb[:, dk, :],
                            start=(dk == 0), stop=(dk == ND - 1))

            def emit_xp_replay(h):
                # tiles 2,3 of half h, replayed from SBUF back-to-back
                for k in (4 * h + 2, 4 * h + 3):
                    xp_ps[k] = ps.tile([128, PCOLS], F32, tag="xp",
                                       bufs=2, name=f"xp_{k}")
                    for dk in range(ND):
                        nc.tensor.matmul(
                            xp_ps[k],
                            xT_sb[:, dk, k * 128:(k + 1) * 128],
                            P_sb[:, dk, :],
                            start=(dk == 0), stop=(dk == ND - 1))

            def emit_copies(lo):
                # drain xp psum (pair lo, lo+1) to SBUF as fp16
                for k in (lo, lo + 1):
                    nc.vector.tensor_copy(xp_sb[:, k, :], xp_ps[k])

            def emit_cum(lo):
                # tile pair (lo, lo+1), one psum bank: U^T over both tiles'
                # 512 cols at once; odd tile adds ONES^T xp_even; per-tile
                # sel32 carry broadcast. Row 127 of the odd half is the next
                # pair's carry.
                p = lo // 2
                cum_ps = ps.tile([128, 2, NCUM], F32, tag="cum", bufs=2)
                nc.tensor.matmul(cum_ps, U_sb, xp_sb[:, lo:lo + 2, CUM0:],
                                 start=True, stop=False)
                nc.tensor.matmul(cum_ps[:, 1, :], ONES_sb,
                                 xp_sb[:, lo, CUM0:],
                                 start=False, stop=False)
                nc.tensor.matmul(cum_ps[:, 0, :], sel32_sb,
                                 carry_sb[:, p, :], start=False, stop=False)
                nc.tensor.matmul(cum_ps[:, 1, :], sel32_sb,
                                 carry_sb[:, p, :], start=False, stop=True)
                if lo < NT - 2:
                    nc.vector.tensor_copy(carry_sb[:, p + 1, :],
                                          cum_ps[96:128, 1, :])
                invb = invc_sb[:, lo:lo + 2, None].broadcast_to(
                    [128, 2, NCUM])
                nc.vector.tensor_mul(cum_sb[:, lo:lo + 2, :], cum_ps, invb)

            def emit_ew(lo, n):
                # G for tiles [lo, lo+n): vector does g1, gpsimd does g2
                xps = xp_sb[:, lo:lo + n, :]
                cms = cum_sb[:, lo:lo + n, :]
                m2 = m2_sb[:, lo:lo + n, :]
                g = G_sb[:, lo:lo + n, :]
                nc.vector.tensor_mul(m2, xps[:, :, 64:128],
                                     cms[:, :, 64:128])
                nc.vector.tensor_mul(g[:, :, 0:64], xps[:, :, 0:64],
                                     cms[:, :, 0:64])
                nc.vector.tensor_add(g[:, :, 0:64], g[:, :, 0:64], m2)
                nc.gpsimd.tensor_mul(g[:, :, 64:128], xps[:, :, 128:192],
                                     cms[:, :, 128:192])
                nc.gpsimd.tensor_mul(g[:, :, 64:128], g[:, :, 64:128],
                                     cms[:, :, 192:256])

            def emit_tp(lo, n):
                for k in range(lo, lo + n):
                    gt_ps = ps.tile([128, 128], F16, tag="tp", bufs=1)
                    nc.tensor.transpose(gt_ps, G_sb[:, k, :], IDN_sb)
                    nc.scalar.copy(GT_sb[:, k * 128:(k + 1) * 128], gt_ps)

            outV = outT.rearrange("(a p) t -> p a t", p=128)

            def emit_final(t0, tw, qs):
                # out[:, t0*128 : (t0+tw)*128] = A.T^T @ GT cols; out DMA per
                # 2-dk rows on alternating queues from qs
                w = tw * 128
                for dk in range(ND):
                    o_ps = ps.tile([128, 512], F32, tag="fin", bufs=3)
                    nc.tensor.matmul(o_ps[:, 0:w],
                                     AT_sb[:, dk * 128:(dk + 1) * 128],
                                     GT_sb[:, t0 * 128:t0 * 128 + w],
                                     start=True, stop=True)
                    co = (t0 % 4) * 128
                    nc.vector.tensor_copy(o_stage[:, dk, co:co + w // 2],
                                          o_ps[:, 0:w // 2])
                    nc.scalar.copy(o_stage[:, dk, co + w // 2:co + w],
                                   o_ps[:, w // 2:w])
                    if dk % 2 == 1:
                        qo = qs[(dk // 2) % len(qs)]
                        qo.dma_start(
                            out=outV[:, dk - 1:dk + 1,
                                     t0 * 128:t0 * 128 + w],
                            in_=o_stage[:, dk - 1:dk + 1, co:co + w])

            # ---- schedule ----
            for j in range(4):
                emit_xp_stream(0, j)
            emit_copies(0)
            emit_xp_replay(0)
            emit_copies(2)
            emit_cum(0)
            emit_cum(2)
            emit_xp_stream(1, 0)
            emit_xp_stream(1, 1)
            emit_ew(0, 4)
            emit_tp(0, 4)
            emit_final(0, 4, (nc.sync, nc.scalar))
            emit_xp_stream(1, 2)
            emit_xp_stream(1, 3)
            emit_copies(4)
            emit_cum(4)
            emit_xp_replay(1)
            emit_ew(4, 2)
            emit_tp(4, 2)
            emit_final(4, 2, (nc.sync,))
            emit_copies(6)
            emit_cum(6)
            emit_ew(6, 2)
            emit_tp(6, 2)
            emit_final(6, 2, (nc.scalar,))

    nc.finalize()
    return nc


_NC = None


def _get_nc():
    global _NC
    if _NC is None:
        _NC = build_nc()
    return _NC


def _fold_weights(WQ, WK, WO, Winv, U_b, V_b, W_b, U_t, V_t, W_t, X_t,
                  alpha_bi, alpha_tri):
    f8 = np.float64
    WQ, WK, WO, Winv = (np.asarray(m) for m in (WQ, WK, WO, Winv))
    U_b, V_b, W_b = (np.asarray(m) for m in (U_b, V_b, W_b))
    U_t, V_t, W_t, X_t = (np.asarray(m) for m in (U_t, V_t, W_t, X_t))
    WQt = WQ.astype(f8).T
    WKt = WK.astype(f8).T
    Winvt = Winv.astype(f8).T
    P = np.concatenate([
        WQt @ V_b.astype(f8),
        float(alpha_bi) * (WQt @ (Winvt @ W_b.astype(f8))),
        WQt @ V_t.astype(f8),
        WKt @ W_b.astype(f8),
        WKt @ (Winvt @ V_b.astype(f8)),
        WKt @ W_t.astype(f8),
        X_t.astype(f8),
    ], axis=1).astype(np.float32)
    A = np.concatenate([
        WO.astype(f8) @ U_b.astype(f8),
        float(alpha_tri) * (WO.astype(f8) @ U_t.astype(f8)),
    ], axis=1).astype(np.float32)
    return P, A


def _make_consts(h):
    counts = np.arange(h * TH + 1, (h + 1) * TH + 1, dtype=np.float64)
    invc = np.ascontiguousarray(
        (1.0 / counts).astype(np.float32).reshape(NT, 128).T)
    return invc


def make_in_maps(x, P, A):
    AT = np.ascontiguousarray(A.T.astype(np.float16))
    PSw = np.ascontiguousarray(
        P.reshape(ND, 128, PCOLS).transpose(1, 0, 2).astype(np.float16))
    P64 = P.astype(np.float64)
    in_maps = []
    for core in range(8):
        b, h = core // 2, core % 2
        xh = x[b, h * TH:(h + 1) * TH, :].astype(np.float16)
        xSw = np.ascontiguousarray(xh.T.reshape(ND, 128, TH).transpose(1, 0, 2))
        if h == 1:
            sx = x[b, :TH, :].sum(axis=0, dtype=np.float64)
            sxP = (sx @ P64[:, CUM0:]).astype(np.float16)[None, :]
        else:
            sxP = np.zeros((1, NCUM), np.float16)
        invc = _make_consts(h)
        in_maps.append(dict(xS=xSw, PS=PSw, AT=AT, sxP=sxP, invc=invc))
    return in_maps


def kernel(x, WQ, WK, WO, Winv, U_b, V_b, W_b, bias_b,
           U_t, V_t, W_t, X_t, bias_t, alpha_bi, alpha_tri):
    x = np.asarray(x, dtype=np.float32)
    P, A = _fold_weights(WQ, WK, WO, Winv, U_b, V_b, W_b,
                         U_t, V_t, W_t, X_t, alpha_bi, alpha_tri)
    in_maps = make_in_maps(x, P, A)

    res = run_bass_kernel_spmd(_get_nc(), in_maps, core_ids=list(range(8)))

    out = np.empty((B, T, D), np.float32)
    for core in range(8):
        b, h = core // 2, core % 2
        out[b, h * TH:(h + 1) * TH, :] = \
            res.results[core]["outT"].T.astype(np.float32)

    # constant bias term (zero for the given inputs, kept for fidelity)
    bias_out = ((1.0 + float(alpha_bi)) * np.asarray(bias_b, np.float64)
                + float(alpha_tri) * np.asarray(bias_t, np.float64)) \
        @ np.asarray(WO, np.float64).T
    if np.any(bias_out):
        out += bias_out.astype(np.float32)[None, None, :]
    return out


# revision 3
# speedup vs baseline: 1.0250x; 1.0250x over previous
"""CausalBiTrilinearBCNAttention Trainium2 kernel.

Math (see _fold_weights): xp = x @ P (D x 448), causal-cumsum the last 256
columns, G = [g1|g2] from elementwise products, out = G @ A.T.

Sharding: 8 cores = 4 batches x 2 T-halves (TH=1024 tokens/core); the
second half's cumsum seed sxP = (sum_t x[:TH]) @ P_cum comes from the host
as a [1,256] fp16 row (zeros for first halves).

Dataflow: host-preswizzled DMA layouts (every descriptor contiguous per
partition), inputs streamed in consumption order across both HWDGE queues,
tokens processed in two 512-token waves so wave 0's tail (cumsum/ew/final/
output) overlaps wave 1's x DMA + xp matmuls. 2 PSUM banks hold xp
accumulators (tiles 0-1 of each wave stream behind the DMA, tiles 2-3
replay from SBUF). Cumsum carries: tile pair (2j,2j+1) accumulates
U^T xp_even / +ones x carry / +ONES^T xp_even for the odd tile, so row 127
of the odd tile's PSUM IS the next pair's carry - one [1,256] copy per
pair, no block-sum matmuls, no cross-engine chain. Cumsum normalization
(1/count) is a per-pair free-dim-broadcast multiply on the vector engine;
PSUM drains are balanced across vector (xp, cum, final lo-half, carry)
and scalar (GT, final hi-half); output staged per 2-dk rows and DMA'd on
both queues as soon as each slice drains.
"""

import numpy as np

import concourse.bass as bass
import concourse.tile as tile
from concourse import bacc, mybir
from concourse.bass_utils import run_bass_kernel_spmd

B, T, D, R = 4, 2048, 1024, 64
TH = T // 2          # tokens per core
NT = TH // 128       # 8 token tiles per core
ND = D // 128        # 8 d chunks
PCOLS = 448          # 7 * R
CUM0 = 192           # start of cumsum group in P's columns
NCUM = 256           # cumsum group width

F32 = mybir.dt.float32
F16 = mybir.dt.float16


def build_nc():
    nc = bacc.Bacc(None, target_bir_lowering=False)

    xS = nc.dram_tensor("xS", [128, ND, TH], F16, kind="ExternalInput")
    PS = nc.dram_tensor("PS", [128, ND, PCOLS], F16, kind="ExternalInput")
    AT = nc.dram_tensor("AT", [128, D], F16, kind="ExternalInput")
    sxP = nc.dram_tensor("sxP", [1, NCUM], F16, kind="ExternalInput")
    invc = nc.dram_tensor("invc", [128, NT], F32, kind="ExternalInput")
    outT = nc.dram_tensor("outT", [D, TH], F16, kind="ExternalOutput")

    from concourse.masks import make_identity, make_upper_triangular

    with tile.TileContext(nc) as tc:
        with tc.tile_pool(name="consts", bufs=1) as consts, \
             tc.tile_pool(name="big", bufs=1) as big, \
             tc.tile_pool(name="ps", bufs=2, space="PSUM") as ps:

            # ---- SBUF tiles ----
            xT_sb = big.tile([128, ND, TH], F16)
            P_sb = consts.tile([128, ND, PCOLS], F16)
            AT_sb = consts.tile([128, D], F16)
            invc_sb = consts.tile([128, NT], F32)
            carry_sb = big.tile([32, 4, NCUM], F16)  # carry on row 31
            xp_sb = big.tile([128, NT, PCOLS], F16)
            cum_sb = big.tile([128, NT, NCUM], F16)
            G_sb = big.tile([128, NT, 128], F16)
            GT_sb = big.tile([128, TH], F16)
            m2_sb = big.tile([128, NT, 64], F16)
            o_stage = big.tile([128, ND, 512], F16)
            warm_sb = consts.tile([128, 448], F16)

            # ---- DMA issue: consumption order across both queues ----
            # q1 (sync):   sxP P01 xh0_01 P45 xh0_45 xh1_01 xh1_45 [+out]
            # q10 (scalar):invc P23 xh0_23 P67 xh0_67 xh1_23 xh1_67 AT [+out]
            nc.vector.memset(carry_sb, 0.0)
            nc.sync.dma_start(out=carry_sb[31:32, 0, :], in_=sxP[:, :])
            nc.scalar.dma_start(out=invc_sb, in_=invc[:, :])
            nc.sync.dma_start(out=P_sb[:, 0:2, :], in_=PS[:, 0:2, :])
            nc.scalar.dma_start(out=P_sb[:, 2:4, :], in_=PS[:, 2:4, :])
            nc.sync.dma_start(out=xT_sb[:, 0:2, 0:512], in_=xS[:, 0:2, 0:512])
            nc.scalar.dma_start(out=xT_sb[:, 2:4, 0:512],
                                in_=xS[:, 2:4, 0:512])
            nc.sync.dma_start(out=P_sb[:, 4:6, :], in_=PS[:, 4:6, :])
            nc.scalar.dma_start(out=P_sb[:, 6:8, :], in_=PS[:, 6:8, :])
            nc.sync.dma_start(out=xT_sb[:, 4:6, 0:512], in_=xS[:, 4:6, 0:512])
            nc.scalar.dma_start(out=xT_sb[:, 6:8, 0:512],
                                in_=xS[:, 6:8, 0:512])
            nc.sync.dma_start(out=xT_sb[:, 0:2, 512:1024],
                              in_=xS[:, 0:2, 512:1024])
            nc.scalar.dma_start(out=xT_sb[:, 2:4, 512:1024],
                                in_=xS[:, 2:4, 512:1024])
            nc.sync.dma_start(out=xT_sb[:, 4:6, 512:1024],
                              in_=xS[:, 4:6, 512:1024])
            nc.scalar.dma_start(out=xT_sb[:, 6:8, 512:1024],
                                in_=xS[:, 6:8, 512:1024])
            nc.scalar.dma_start(out=AT_sb, in_=AT[:, :])

            # ---- constants on idle engines ----
            nc.vector.memset(warm_sb, 0.0)
            U_sb = consts.tile([128, 128], F16)
            make_upper_triangular(nc, U_sb, val=1.0, diag=True)
            IDN_sb = consts.tile([128, 128], F16)
            make_identity(nc, IDN_sb)
            ONES_sb = consts.tile([128, 128], F16)
            nc.gpsimd.memset(ONES_sb, 1.0)
            # sel32: [32,128] with row 31 all-ones -- sel32^T @ carry32
            # broadcasts carry row 31 to all 128 output partitions
            sel32_sb = consts.tile([32, 128], F16)
            nc.gpsimd.memset(sel32_sb, 0.0)
            nc.gpsimd.affine_select(
                out=sel32_sb, in_=sel32_sb,
                compare_op=mybir.AluOpType.not_equal, fill=1.0,
                base=-31, pattern=[[0, 128]], channel_multiplier=1)

            # ---- PE warmup: ramp the PE while DMA streams in ----
            warm_ps = ps.tile([128, 512], F32, tag="fin", bufs=3)
            for i in range(14):
                nc.tensor.matmul(warm_ps[:, 0:448], warm_sb[:, 0:128],
                                 warm_sb, start=True, stop=True)
            # preload the scalar-engine activation table off the critical path
            actw_sb = consts.tile([128, 8], F16)
            nc.scalar.activation(actw_sb, warm_sb[:, 0:8],
                                 mybir.ActivationFunctionType.Copy)

            xp_ps = [None] * NT

            def emit_xp_stream(h, j):
                # xp matmuls for dk pair j of half h, first two tiles only
                for dk in (2 * j, 2 * j + 1):
                    for k in (4 * h, 4 * h + 1):
                        if dk == 0:
                            xp_ps[k] = ps.tile([128, PCOLS], F32, tag="xp",
                                               bufs=2, name=f"xp_{k}")
                        nc.tensor.matmul(
                            xp_ps[k],
                            xT_sb[:, dk, k * 128:(k + 1) * 128],
                            P_sb[:, dk, :],
                            start=(dk == 0), stop=(dk == ND - 1))

            def emit_xp_replay(h):
                # tiles 2,3 of half h, replayed from SBUF back-to-back
                for k in (4 * h + 2, 4 * h + 3):
                    xp_ps[k] = ps.tile([128, PCOLS], F32, tag="xp",
                                       bufs=2, name=f"xp_{k}")
                    for dk in range(ND):
                        nc.tensor.matmul(
                            xp_ps[k],
                            xT_sb[:, dk, k * 128:(k + 1) * 128],
                            P_sb[:, dk, :],
                            start=(dk == 0), stop=(dk == ND - 1))

            def emit_copies(lo):
                # drain xp psum (pair lo, lo+1) to SBUF as fp16
                for k in (lo, lo + 1):
                    nc.vector.tensor_copy(xp_sb[:, k, :], xp_ps[k])

            def emit_cum(lo):
                # tile pair (lo, lo+1), one psum bank: U^T over both tiles'
                # 512 cols at once; odd tile adds ONES^T xp_even; per-tile
                # sel32 carry broadcast. Row 127 of the odd half is the next
                # pair's carry.
                p = lo // 2
                cum_ps = ps.tile([128, 2, NCUM], F32, tag="cum", bufs=2)
                nc.tensor.matmul(cum_ps, U_sb, xp_sb[:, lo:lo + 2, CUM0:],
                                 start=True, stop=False)
                nc.tensor.matmul(cum_ps[:, 1, :], ONES_sb,
                                 xp_sb[:, lo, CUM0:],
                                 start=False, stop=False)
                nc.tensor.matmul(cum_ps[:, 0, :], sel32_sb,
                                 carry_sb[:, p, :], start=False, stop=False)
                nc.tensor.matmul(cum_ps[:, 1, :], sel32_sb,
                                 carry_sb[:, p, :], start=False, stop=True)
                if lo < NT - 2:
                    nc.vector.tensor_copy(carry_sb[:, p + 1, :],
                                          cum_ps[96:128, 1, :])
                invb = invc_sb[:, lo:lo + 2, None].broadcast_to(
                    [128, 2, NCUM])
                nc.vector.tensor_mul(cum_sb[:, lo:lo + 2, :], cum_ps, invb)

            def emit_ew(lo, n):
                # G for tiles [lo, lo+n): vector does g1, gpsimd does g2
                xps = xp_sb[:, lo:lo + n, :]
                cms = cum_sb[:, lo:lo + n, :]
                m2 = m2_sb[:, lo:lo + n, :]
                g = G_sb[:, lo:lo + n, :]
                nc.vector.tensor_mul(m2, xps[:, :, 64:128],
                                     cms[:, :, 64:128])
                nc.vector.tensor_mul(g[:, :, 0:64], xps[:, :, 0:64],
                                     cms[:, :, 0:64])
                nc.vector.tensor_add(g[:, :, 0:64], g[:, :, 0:64], m2)
                nc.gpsimd.tensor_mul(g[:, :, 64:128], xps[:, :, 128:192],
                                     cms[:, :, 128:192])
                nc.gpsimd.tensor_mul(g[:, :, 64:128], g[:, :, 64:128],
                                     cms[:, :, 192:256])

            def emit_tp(lo, n):
                for k in range(lo, lo + n):
                    gt_ps = ps.tile([128, 128], F16, tag="tp", bufs=1)
                    nc.tensor.transpose(gt_ps, G_sb[:, k, :], IDN_sb)
                    nc.scalar.copy(GT_sb[:, k * 128:(k + 1) * 128], gt_ps)

            outV = outT.rearrange("(a p) t -> p a t", p=128)

            def emit_final(t0, tw, qs):
                # out[:, t0*128 : (t0+tw)*128] = A.T^T @ GT cols; out DMA per
                # 2-dk rows on alternating queues from qs
                w = tw * 128
                for dk in range(ND):
                    o_ps = ps.tile([128, 512], F32, tag="fin", bufs=3)
                    nc.tensor.matmul(o_ps[:, 0:w],
                                     AT_sb[:, dk * 128:(dk + 1) * 128],
                                     GT_sb[:, t0 * 128:t0 * 128 + w],
                                     start=True, stop=True)
                    co = (t0 % 4) * 128
                    nc.vector.tensor_copy(o_stage[:, dk, co:co + w // 2],
                                          o_ps[:, 0:w // 2])
                    nc.scalar.copy(o_stage[:, dk, co + w // 2:co + w],
                                   o_ps[:, w // 2:w])
                    if dk % 2 == 1:
                        qo = qs[(dk // 2) % len(qs)]
                        qo.dma_start(
                            out=outV[:, dk - 1:dk + 1,
                                     t0 * 128:t0 * 128 + w],
                            in_=o_stage[:, dk - 1:dk + 1, co:co + w])

            # ---- schedule ----
            for j in range(4):
                emit_xp_stream(0, j)
            emit_copies(0)
            emit_xp_replay(0)
            emit_copies(2)
            emit_cum(0)
            emit_cum(2)
            emit_xp_stream(1, 0)
            emit_xp_stream(1, 1)
            emit_ew(0, 4)
            emit_tp(0, 4)
            emit_final(0, 4, (nc.sync, nc.scalar))
            emit_xp_stream(1, 2)
            emit_xp_stream(1, 3)
            emit_copies(4)
            emit_cum(4)
            emit_xp_replay(1)
            emit_ew(4, 2)
            emit_tp(4, 2)
            emit_final(4, 2, (nc.sync,))
            emit_copies(6)
            emit_cum(6)
            emit_ew(6, 2)
            emit_tp(6, 2)
            emit_final(6, 2, (nc.scalar,))

    nc.finalize()
    return nc


_NC = None


def _get_nc():
    global _NC
    if _NC is None:
        _NC = build_nc()
    return _NC


def _fold_weights(WQ, WK, WO, Winv, U_b, V_b, W_b, U_t, V_t, W_t, X_t,
                  alpha_bi, alpha_tri):
    f8 = np.float64
    WQ, WK, WO, Winv = (np.asarray(m) for m in (WQ, WK, WO, Winv))
    U_b, V_b, W_b = (np.asarray(m) for m in (U_b, V_b, W_b))
    U_t, V_t, W_t, X_t = (np.asarray(m) for m in (U_t, V_t, W_t, X_t))
    WQt = WQ.astype(f8).T
    WKt = WK.astype(f8).T
    Winvt = Winv.astype(f8).T
    P = np.concatenate([
        WQt @ V_b.astype(f8),
        float(alpha_bi) * (WQt @ (Winvt @ W_b.astype(f8))),
        WQt @ V_t.astype(f8),
        WKt @ W_b.astype(f8),
        WKt @ (Winvt @ V_b.astype(f8)),
        WKt @ W_t.astype(f8),
        X_t.astype(f8),
    ], axis=1).astype(np.float32)
    A = np.concatenate([
        WO.astype(f8) @ U_b.astype(f8),
        float(alpha_tri) * (WO.astype(f8) @ U_t.astype(f8)),
    ], axis=1).astype(np.float32)
    return P, A


def _make_consts(h):
    counts = np.arange(h * TH + 1, (h + 1) * TH + 1, dtype=np.float64)
    invc = np.ascontiguousarray(
        (1.0 / counts).astype(np.float32).reshape(NT, 128).T)
    return invc


def make_in_maps(x, P, A):
    AT = np.ascontiguousarray(A.T.astype(np.float16))
    PSw = np.ascontiguousarray(
        P.reshape(ND, 128, PCOLS).transpose(1, 0, 2).astype(np.float16))
    P64 = P.astype(np.float64)
    in_maps = []
    for core in range(8):
        b, h = core // 2, core % 2
        xh = x[b, h * TH:(h + 1) * TH, :].astype(np.float16)
        xSw = np.ascontiguousarray(xh.T.reshape(ND, 128, TH).transpose(1, 0, 2))
        if h == 1:
            sx = x[b, :TH, :].sum(axis=0, dtype=np.float64)
            sxP = (sx @ P64[:, CUM0:]).astype(np.float16)[None, :]
        else:
            sxP = np.zeros((1, NCUM), np.float16)
        invc = _make_consts(h)
        in_maps.append(dict(xS=xSw, PS=PSw, AT=AT, sxP=sxP, invc=invc))
    return in_maps


def kernel(x, WQ, WK, WO, Winv, U_b, V_b, W_b, bias_b,
           U_t, V_t, W_t, X_t, bias_t, alpha_bi, alpha_tri):
    x = np.asarray(x, dtype=np.float32)
    P, A = _fold_weights(WQ, WK, WO, Winv, U_b, V_b, W_b,
                         U_t, V_t, W_t, X_t, alpha_bi, alpha_tri)
    in_maps = make_in_maps(x, P, A)

    res = run_bass_kernel_spmd(_get_nc(), in_maps, core_ids=list(range(8)))

    out = np.empty((B, T, D), np.float32)
    for core in range(8):
        b, h = core // 2, core % 2
        out[b, h * TH:(h + 1) * TH, :] = \
            res.results[core]["outT"].T.astype(np.float32)

    # constant bias term (zero for the given inputs, kept for fidelity)
    bias_out = ((1.0 + float(alpha_bi)) * np.asarray(bias_b, np.float64)
                + float(alpha_tri) * np.asarray(bias_t, np.float64)) \
        @ np.asarray(WO, np.float64).T
    if np.any(bias_out):
        out += bias_out.astype(np.float32)[None, None, :]
    return out


# revision 4
# speedup vs baseline: 1.0478x; 1.0222x over previous
"""CausalBiTrilinearBCNAttention Trainium2 kernel, v3.

Math (see _fold_weights): xp = x @ P (D x 448), causal-cumsum the last 256
columns, G = [g1|g2] from elementwise products, out = G @ A.T.

Sharding: 8 cores = 4 batches x 2 T-halves (TH=1024 tokens/core); the
second half's cumsum seed sxP = (sum_t x[:TH]) @ P_cum comes from the host
as a [1,256] fp16 row (zeros for first halves).

Dataflow: host-preswizzled DMA layouts (every descriptor contiguous per
partition), inputs streamed in consumption order across both HWDGE queues,
tokens processed in two 512-token waves so wave 0's tail (cumsum/ew/final/
output) overlaps wave 1's x DMA + xp matmuls. 2 PSUM banks hold xp
accumulators (tiles 0-1 of each wave stream behind the DMA, tiles 2-3
replay from SBUF). Cumsum carries: tile pair (2j,2j+1) accumulates
U^T xp_even / +ones x carry / +ONES^T xp_even for the odd tile, so row 127
of the odd tile's PSUM IS the next pair's carry - one [1,256] copy per
pair, no block-sum matmuls, no cross-engine chain.
"""

import numpy as np

import concourse.bass as bass
import concourse.tile as tile
from concourse import bacc, mybir
from concourse.bass_utils import run_bass_kernel_spmd

B, T, D, R = 4, 2048, 1024, 64
TH = T // 2          # tokens per core
NT = TH // 128       # 8 token tiles per core
ND = D // 128        # 8 d chunks
PCOLS = 448          # 7 * R
CUM0 = 192           # start of cumsum group in P's columns
NCUM = 256           # cumsum group width

F32 = mybir.dt.float32
F16 = mybir.dt.float16


def build_nc():
    nc = bacc.Bacc(None, target_bir_lowering=False)

    xS = nc.dram_tensor("xS", [128, ND, TH], F16, kind="ExternalInput")
    PS = nc.dram_tensor("PS", [128, ND, PCOLS], F16, kind="ExternalInput")
    AT = nc.dram_tensor("AT", [128, D], F16, kind="ExternalInput")
    sxP = nc.dram_tensor("sxP", [1, NCUM], F16, kind="ExternalInput")
    invc = nc.dram_tensor("invc", [128, NT], F32, kind="ExternalInput")
    outT = nc.dram_tensor("outT", [D, TH], F16, kind="ExternalOutput")

    from concourse.masks import make_identity, make_upper_triangular

    with tile.TileContext(nc) as tc:
        with tc.tile_pool(name="consts", bufs=1) as consts, \
             tc.tile_pool(name="big", bufs=1) as big, \
             tc.tile_pool(name="ps", bufs=2, space="PSUM") as ps:

            # ---- SBUF tiles ----
            xT_sb = big.tile([128, ND, TH], F16)
            P_sb = consts.tile([128, ND, PCOLS], F16)
            AT_sb = consts.tile([128, D], F16)
            invc_sb = consts.tile([128, NT], F32)
            carry_sb = big.tile([32, 4, NCUM], F16)  # carry on row 31
            xp_sb = big.tile([128, NT, PCOLS], F16)
            cum_sb = big.tile([128, NT, NCUM], F16)
            G_sb = big.tile([128, NT, 128], F16)
            GT_sb = big.tile([128, TH], F16)
            m2_sb = big.tile([128, NT, 64], F16)
            o_stage = big.tile([128, ND, 512], F16)
            warm_sb = consts.tile([128, 448], F16)

            # ---- DMA issue: consumption order across both queues ----
            # q1 (sync):   sxP P01 xh0_01 P45 xh0_45 xh1_01 xh1_45 [+out]
            # q10 (scalar):invc P23 xh0_23 P67 xh0_67 xh1_23 xh1_67 AT [+out]
            nc.vector.memset(carry_sb, 0.0)
            nc.sync.dma_start(out=carry_sb[31:32, 0, :], in_=sxP[:, :])
            nc.scalar.dma_start(out=invc_sb, in_=invc[:, :])
            nc.sync.dma_start(out=P_sb[:, 0:2, :], in_=PS[:, 0:2, :])
            nc.scalar.dma_start(out=P_sb[:, 2:4, :], in_=PS[:, 2:4, :])
            nc.sync.dma_start(out=xT_sb[:, 0:2, 0:512], in_=xS[:, 0:2, 0:512])
            nc.scalar.dma_start(out=xT_sb[:, 2:4, 0:512],
                                in_=xS[:, 2:4, 0:512])
            nc.sync.dma_start(out=P_sb[:, 4:6, :], in_=PS[:, 4:6, :])
            nc.scalar.dma_start(out=P_sb[:, 6:8, :], in_=PS[:, 6:8, :])
            nc.sync.dma_start(out=xT_sb[:, 4:6, 0:512], in_=xS[:, 4:6, 0:512])
            nc.scalar.dma_start(out=xT_sb[:, 6:8, 0:512],
                                in_=xS[:, 6:8, 0:512])
            nc.sync.dma_start(out=xT_sb[:, 0:2, 512:1024],
                              in_=xS[:, 0:2, 512:1024])
            nc.scalar.dma_start(out=xT_sb[:, 2:4, 512:1024],
                                in_=xS[:, 2:4, 512:1024])
            nc.sync.dma_start(out=xT_sb[:, 4:6, 512:1024],
                              in_=xS[:, 4:6, 512:1024])
            nc.scalar.dma_start(out=xT_sb[:, 6:8, 512:1024],
                                in_=xS[:, 6:8, 512:1024])
            nc.scalar.dma_start(out=AT_sb, in_=AT[:, :])

            # ---- constants on idle engines ----
            nc.vector.memset(warm_sb, 0.0)
            U_sb = consts.tile([128, 128], F16)
            make_upper_triangular(nc, U_sb, val=1.0, diag=True)
            IDN_sb = consts.tile([128, 128], F16)
            make_identity(nc, IDN_sb)
            ONES_sb = consts.tile([128, 128], F16)
            nc.gpsimd.memset(ONES_sb, 1.0)
            # sel32: [32,128] with row 31 all-ones -- sel32^T @ carry32
            # broadcasts carry row 31 to all 128 output partitions
            sel32_sb = consts.tile([32, 128], F16)
            nc.gpsimd.memset(sel32_sb, 0.0)
            nc.gpsimd.affine_select(
                out=sel32_sb, in_=sel32_sb,
                compare_op=mybir.AluOpType.not_equal, fill=1.0,
                base=-31, pattern=[[0, 128]], channel_multiplier=1)

            # ---- PE warmup: ramp the PE while DMA streams in ----
            warm_ps = ps.tile([128, 512], F32, tag="fin", bufs=3)
            for i in range(40):
                nc.tensor.matmul(warm_ps[:, 0:224], warm_sb[:, 0:128],
                                 warm_sb[:, 0:224], start=True, stop=True)
            # preload the scalar-engine activation table off the critical path
            actw_sb = consts.tile([128, 8], F16)
            nc.scalar.activation(actw_sb, warm_sb[:, 0:8],
                                 mybir.ActivationFunctionType.Copy)

            xp_ps = [None] * NT

            def emit_xp_stream(h, j):
                # xp matmuls for dk pair j of half h, first two tiles only
                for dk in (2 * j, 2 * j + 1):
                    for k in (4 * h, 4 * h + 1):
                        if dk == 0:
                            xp_ps[k] = ps.tile([128, PCOLS], F32, tag="xp",
                                               bufs=2, name=f"xp_{k}")
                        nc.tensor.matmul(
                            xp_ps[k],
                            xT_sb[:, dk, k * 128:(k + 1) * 128],
                            P_sb[:, dk, :],
                            start=(dk == 0), stop=(dk == ND - 1))

            def emit_xp_replay(h):
                # tiles 2,3 of half h, replayed from SBUF back-to-back
                for k in (4 * h + 2, 4 * h + 3):
                    xp_ps[k] = ps.tile([128, PCOLS], F32, tag="xp",
                                       bufs=2, name=f"xp_{k}")
                    for dk in range(ND):
                        nc.tensor.matmul(
                            xp_ps[k],
                            xT_sb[:, dk, k * 128:(k + 1) * 128],
                            P_sb[:, dk, :],
                            start=(dk == 0), stop=(dk == ND - 1))

            def emit_copies(lo):
                # drain xp psum (pair lo, lo+1) to SBUF as fp16
                for k in (lo, lo + 1):
                    nc.vector.tensor_copy(xp_sb[:, k, :], xp_ps[k])

            def emit_cum(lo):
                # tile pair (lo, lo+1), one psum bank: U^T over both tiles'
                # 512 cols at once; odd tile adds ONES^T xp_even; per-tile
                # sel32 carry broadcast. Row 127 of the odd half is the next
                # pair's carry.
                p = lo // 2
                cum_ps = ps.tile([128, 2, NCUM], F32, tag="cum", bufs=2)
                nc.tensor.matmul(cum_ps, U_sb, xp_sb[:, lo:lo + 2, CUM0:],
                                 start=True, stop=False)
                nc.tensor.matmul(cum_ps[:, 1, :], ONES_sb,
                                 xp_sb[:, lo, CUM0:],
                                 start=False, stop=False)
                nc.tensor.matmul(cum_ps[:, 0, :], sel32_sb,
                                 carry_sb[:, p, :], start=False, stop=False)
                nc.tensor.matmul(cum_ps[:, 1, :], sel32_sb,
                                 carry_sb[:, p, :], start=False, stop=True)
                if lo < NT - 2:
                    nc.vector.tensor_copy(carry_sb[:, p + 1, :],
                                          cum_ps[96:128, 1, :])
                invb = invc_sb[:, lo:lo + 2, None].broadcast_to(
                    [128, 2, NCUM])
                nc.vector.tensor_mul(cum_sb[:, lo:lo + 2, :], cum_ps, invb)

            def emit_ew(lo, n):
                # G for tiles [lo, lo+n): vector does g1, gpsimd does g2
                xps = xp_sb[:, lo:lo + n, :]
                cms = cum_sb[:, lo:lo + n, :]
                m2 = m2_sb[:, lo:lo + n, :]
                g = G_sb[:, lo:lo + n, :]
                nc.vector.tensor_mul(m2, xps[:, :, 64:128],
                                     cms[:, :, 64:128])
                nc.vector.tensor_mul(g[:, :, 0:64], xps[:, :, 0:64],
                                     cms[:, :, 0:64])
                nc.vector.tensor_add(g[:, :, 0:64], g[:, :, 0:64], m2)
                nc.gpsimd.tensor_mul(g[:, :, 64:128], xps[:, :, 128:192],
                                     cms[:, :, 128:192])
                nc.gpsimd.tensor_mul(g[:, :, 64:128], g[:, :, 64:128],
                                     cms[:, :, 192:256])

            def emit_tp(lo, n):
                for k in range(lo, lo + n):
                    gt_ps = ps.tile([128, 128], F16, tag="tp", bufs=1)
                    nc.tensor.transpose(gt_ps, G_sb[:, k, :], IDN_sb)
                    nc.scalar.copy(GT_sb[:, k * 128:(k + 1) * 128], gt_ps)

            outV = outT.rearrange("(a p) t -> p a t", p=128)

            def emit_final(t0, tw, qs, fine_tail=False):
                # out[:, t0*128 : (t0+tw)*128] = A.T^T @ GT cols; single-
                # engine copy per dk (alternating vector/scalar); out DMA
                # per 2-dk rows, with per-1-dk descriptors at the tail when
                # fine_tail is set so the last transfer is small.
                w = tw * 128
                co = (t0 % 4) * 128
                for dk in range(ND):
                    o_ps = ps.tile([128, 512], F32, tag="fin", bufs=3)
                    nc.tensor.matmul(o_ps[:, 0:w],
                                     AT_sb[:, dk * 128:(dk + 1) * 128],
                                     GT_sb[:, t0 * 128:t0 * 128 + w],
                                     start=True, stop=True)
                    if dk % 2 == 0:
                        nc.vector.tensor_copy(o_stage[:, dk, co:co + w],
                                              o_ps[:, 0:w])
                    else:
                        nc.scalar.copy(o_stage[:, dk, co:co + w],
                                       o_ps[:, 0:w])
                    last2 = fine_tail and dk >= ND - 2
                    if last2:
                        qo = qs[dk % len(qs)]
                        qo.dma_start(
                            out=outV[:, dk:dk + 1, t0 * 128:t0 * 128 + w],
                            in_=o_stage[:, dk:dk + 1, co:co + w])
                    elif dk % 2 == 1 and dk < (ND - 2 if fine_tail else ND):
                        qo = qs[(dk // 2) % len(qs)]
                        qo.dma_start(
                            out=outV[:, dk - 1:dk + 1,
                                     t0 * 128:t0 * 128 + w],
                            in_=o_stage[:, dk - 1:dk + 1, co:co + w])

            # ---- schedule ----
            for j in range(4):
                emit_xp_stream(0, j)
            emit_copies(0)
            emit_xp_replay(0)
            emit_copies(2)
            emit_cum(0)
            emit_cum(2)
            emit_xp_stream(1, 0)
            emit_xp_stream(1, 1)
            emit_ew(0, 4)
            emit_tp(0, 4)
            emit_final(0, 4, (nc.sync, nc.scalar))
            emit_xp_stream(1, 2)
            emit_xp_stream(1, 3)
            emit_copies(4)
            emit_cum(4)
            emit_xp_replay(1)
            emit_ew(4, 2)
            emit_tp(4, 2)
            emit_final(4, 2, (nc.sync,))
            emit_copies(6)
            emit_cum(6)
            emit_ew(6, 2)
            emit_tp(6, 2)
            emit_final(6, 2, (nc.scalar, nc.sync), fine_tail=True)

    nc.finalize()
    return nc


_NC = None


def _get_nc():
    global _NC
    if _NC is None:
        _NC = build_nc()
    return _NC


def _fold_weights(WQ, WK, WO, Winv, U_b, V_b, W_b, U_t, V_t, W_t, X_t,
                  alpha_bi, alpha_tri):
    f8 = np.float64
    WQ, WK, WO, Winv = (np.asarray(m) for m in (WQ, WK, WO, Winv))
    U_b, V_b, W_b = (np.asarray(m) for m in (U_b, V_b, W_b))
    U_t, V_t, W_t, X_t = (np.asarray(m) for m in (U_t, V_t, W_t, X_t))
    WQt = WQ.astype(f8).T
    WKt = WK.astype(f8).T
    Winvt = Winv.astype(f8).T
    P = np.concatenate([
        WQt @ V_b.astype(f8),
        float(alpha_bi) * (WQt @ (Winvt @ W_b.astype(f8))),
        WQt @ V_t.astype(f8),
        WKt @ W_b.astype(f8),
        WKt @ (Winvt @ V_b.astype(f8)),
        WKt @ W_t.astype(f8),
        X_t.astype(f8),
    ], axis=1).astype(np.float32)
    A = np.concatenate([
        WO.astype(f8) @ U_b.astype(f8),
        float(alpha_tri) * (WO.astype(f8) @ U_t.astype(f8)),
    ], axis=1).astype(np.float32)
    return P, A


def _make_consts(h):
    counts = np.arange(h * TH + 1, (h + 1) * TH + 1, dtype=np.float64)
    invc = np.ascontiguousarray(
        (1.0 / counts).astype(np.float32).reshape(NT, 128).T)
    return invc


def make_in_maps(x, P, A):
    AT = np.ascontiguousarray(A.T.astype(np.float16))
    PSw = np.ascontiguousarray(
        P.reshape(ND, 128, PCOLS).transpose(1, 0, 2).astype(np.float16))
    P64 = P.astype(np.float64)
    in_maps = []
    for core in range(8):
        b, h = core // 2, core % 2
        xh = x[b, h * TH:(h + 1) * TH, :].astype(np.float16)
        xSw = np.ascontiguousarray(xh.T.reshape(ND, 128, TH).transpose(1, 0, 2))
        if h == 1:
            sx = x[b, :TH, :].sum(axis=0, dtype=np.float64)
            sxP = (sx @ P64[:, CUM0:]).astype(np.float16)[None, :]
        else:
            sxP = np.zeros((1, NCUM), np.float16)
        invc = _make_consts(h)
        in_maps.append(dict(xS=xSw, PS=PSw, AT=AT, sxP=sxP, invc=invc))
    return in_maps


def kernel(x, WQ, WK, WO, Winv, U_b, V_b, W_b, bias_b,
           U_t, V_t, W_t, X_t, bias_t, alpha_bi, alpha_tri):
    x = np.asarray(x, dtype=np.float32)
    P, A = _fold_weights(WQ, WK, WO, Winv, U_b, V_b, W_b,
                         U_t, V_t, W_t, X_t, alpha_bi, alpha_tri)
    in_maps = make_in_maps(x, P, A)

    res = run_bass_kernel_spmd(_get_nc(), in_maps, core_ids=list(range(8)))

    out = np.empty((B, T, D), np.float32)
    for core in range(8):
        b, h = core // 2, core % 2
        out[b, h * TH:(h + 1) * TH, :] = \
            res.results[core]["outT"].T.astype(np.float32)

    # constant bias term (zero for the given inputs, kept for fidelity)
    bias_out = ((1.0 + float(alpha_bi)) * np.asarray(bias_b, np.float64)
                + float(alpha_tri) * np.asarray(bias_t, np.float64)) \
        @ np.asarray(WO, np.float64).T
    if np.any(bias_out):
        out += bias_out.astype(np.float32)[None, None, :]
    return out
